# revision 15
# baseline (speedup 1.0000x reference)
"""Trainium2 Bass kernel for the DescriptorLoss dual-softmax loss.

Math (per batch element b):
    des1 = p1[b][:, y1, x1]            # [C=256, N=3540]
    des2 = p2[b][:, y2, x2]            # [C, N]
    dist = TEMP * des1.T @ des2        # [N, N]
    loss_b = 2*trace(dist) - sum_m lse_row[m] - sum_n lse_col[n]
    loss   = -(sum_b loss_b) / (B*N)

Sharding: data-parallel over the batch dim, one batch element per
NeuronCore (B == 8 == n_cores).  Host gathers descriptors, quantizes to
fp8e4m3 in DoubleRow block layouts, runs the SPMD program, and combines
the per-core partial sums (loss tolerance 2e-2; fp8 dist + a small
Schraudolph-exp slice keep rel err ~5e-4, validated vs reference).

Per-core structure (two PSUM regions R0=[0,1536), R1=[1536,3540),
pipelined as in the proven baseline; per-tile steady state ~3.7us):
    PE : dist via fp8 DoubleRow matmuls - contraction 256 = 128
         partitions x 2 interleaved rows.  Block layouts keep the pair
         stride small (lhsT [128,28,2,128] blocks per m-tile, rhs
         [128,7,2,512] blocks per 512 columns); large pair strides are
         rejected when >16B-misaligned and run slower.  One matmul per
         512-col chunk instead of a 2-deep k-loop halves the PE
         instruction + ldweights count vs the bf16 baseline.
         Ones-matmul finalize for column sums + partition reductions.
    ACT: exact exp of R0 and of [1536, AW2), accum_out row sums.
    DVE: Schraudolph exp of the tail [AW2, N) (tensor_scalar
         (x*S + B) -> int16 bitcast bf16), its rowsum pass (accum),
         both colacc += exp adds (bf16 2x), and the exact-diag stt.
"""

import numpy as np
import ml_dtypes

B = 8
C = 256
N = 3540
TEMP = 0.2
MT = 128
NT = (N + MT - 1) // MT          # 28 m-tiles (last has 84 rows)
NB = 7                            # rhs 512-col blocks (last holds 468)
MP_PAD = NT * MT                  # 3584, lhsT m padded

R0 = 1536                         # region 0 = [0, R0), region 1 = [R0, N)
AW2 = 3090                        # ACT exp [0, AW2); DVE Schraudolph [AW2, N)

# Schraudolph: i16 = trunc(raw_dot * SCH_S + SCH_B); bitcast bf16
# approximates exp(TEMP * raw_dot).  C = -6.5 calibrated for minimal bias.
SCH_S = TEMP * 128.0 / float(np.log(2.0))
SCH_B = 16256.0 - 6.5

_prog_cache = {}

# (block j, in-block offset, width) chunk lists per region: every PSUM
# output chunk stays inside one 2KB bank, every rhs chunk inside one
# 512-col block.
_R0_CHUNKS = [(0, 0, 512), (1, 0, 512), (2, 0, 512)]
_R1_CHUNKS = [(3, 0, 512), (4, 0, 512), (5, 0, 512), (6, 0, 468)]


def _mm_chunks(width):
    out = []
    off = 0
    while off < width:
        w = min(512, width - off)
        out.append((off, w))
        off += w
    return out


def _build_program():
    import concourse.bacc as bacc
    import concourse.tile as tile
    from concourse import mybir

    dt = mybir.dt
    f32 = dt.float32
    bf16 = dt.bfloat16
    i16 = dt.int16
    fp8 = dt.float8e4
    Exp = mybir.ActivationFunctionType.Exp
    Ln = mybir.ActivationFunctionType.Ln
    MULT = mybir.AluOpType.mult
    ADD = mybir.AluOpType.add
    DR = mybir.MatmulPerfMode.DoubleRow

    nc = bacc.Bacc(
        "TRN2", target_bir_lowering=False, debug=False, num_devices=B)
    d1 = nc.dram_tensor("d1", [MT, NT, 2, MT], fp8, kind="ExternalInput")
    d2 = nc.dram_tensor("d2", [MT, NB, 2, 512], fp8, kind="ExternalInput")
    b1 = nc.dram_tensor("b1", [MT, 2 * N], bf16, kind="ExternalInput")
    b2 = nc.dram_tensor("b2", [MT, 2 * N], bf16, kind="ExternalInput")
    out = nc.dram_tensor("out", [1, 3], f32, kind="ExternalOutput")

    with tile.TileContext(nc) as tc:
        with (
            tc.tile_pool(name="persist", bufs=1) as persist,
            tc.tile_pool(name="etiles", bufs=2) as etiles,
            tc.tile_pool(name="small", bufs=1) as small,
        ):
            d1_sb = persist.tile([MT, NT, 2, MT], fp8, name="d1_sb")
            d2_sb = persist.tile([MT, NB, 2, 512], fp8, name="d2_sb")
            b1_sb = persist.tile([MT, 2 * N], bf16, name="b1_sb")
            b2_sb = persist.tile([MT, 2 * N], bf16, name="b2_sb")

            # fp8 operands first (tile 0 needs all of d2 + head of d1).
            nc.sync.dma_start(out=d2_sb, in_=d2[:, :, :, :])
            nc.scalar.dma_start(out=d1_sb[:, 0:4, :, :], in_=d1[:, 0:4, :, :])
            nc.scalar.dma_start(out=d1_sb[:, 4:NT, :, :],
                                in_=d1[:, 4:NT, :, :])
            # bf16 copies for the diag term.
            nc.sync.dma_start(out=b1_sb, in_=b1[:, :])
            nc.scalar.dma_start(out=b2_sb, in_=b2[:, :])

            colacc = persist.tile([MT, N], bf16, name="colacc")
            nc.vector.memset(colacc, 0.0)

            # rsparts blocks: 0 = ACT R0, 1 = ACT R1-head, 2 = DVE tail.
            # 0.5/0.5/0.0-init: missing rows of the last m-tile sum to 1.0
            # -> Ln contributes 0.
            rsparts = small.tile([MT, 3 * NT], f32, name="rsparts")
            nc.vector.memset(rsparts[:, 0:2 * NT], 0.5)
            nc.vector.memset(rsparts[:, 2 * NT:3 * NT], 0.0)

            ones_bf = small.tile([MT, 1], bf16, name="ones_bf")
            nc.vector.memset(ones_bf, 1.0)
            ones_f32 = small.tile([MT, 1], f32, name="ones_f32")
            nc.vector.memset(ones_f32, 1.0)

            # fin[:,0] = diag partial, fin[:,1] = sum of row-logs partial
            fin = small.tile([MT, 2], f32, name="fin")

            # diag = sum over C of des1*des2 per column: one stt per half,
            # off the critical path (overlaps DMA + PE ramp).
            scratch = persist.tile([MT, N], bf16, name="scratch")
            diag0 = small.tile([MT, 1], f32, name="diag0")
            diag1 = small.tile([MT, 1], f32, name="diag1")
            nc.vector.scalar_tensor_tensor(
                out=scratch, in0=b1_sb[:, 0:N], scalar=1.0,
                in1=b2_sb[:, 0:N], op0=MULT, op1=MULT, accum_out=diag0)
            nc.vector.scalar_tensor_tensor(
                out=scratch, in0=b1_sb[:, N:2 * N], scalar=1.0,
                in1=b2_sb[:, N:2 * N], op0=MULT, op1=MULT, accum_out=diag1)
            nc.vector.tensor_add(fin[:, 0:1], diag0, diag1)

            with (
                tc.tile_pool(name="ps0", bufs=1, space="PSUM") as psp0,
                tc.tile_pool(name="ps1", bufs=1, space="PSUM") as psp1,
            ):
                for t in range(NT):
                    mp = min(MT, N - t * MT)
                    p0 = psp0.tile([MT, R0], f32, tag="p0", name="p0")
                    p1t = psp1.tile([MT, N - R0], f32, tag="p1", name="p1")
                    lhsT = d1_sb[:, t, :, 0:mp]
                    for (j, o, w) in _R0_CHUNKS:
                        g = 512 * j + o
                        nc.tensor.matmul(
                            p0[:mp, g:g + w],
                            lhsT=lhsT, rhs=d2_sb[:, j, :, o:o + w],
                            start=True, stop=True, perf_mode=DR)
                    for (j, o, w) in _R1_CHUNKS:
                        g = 512 * j + o - R0
                        nc.tensor.matmul(
                            p1t[:mp, g:g + w],
                            lhsT=lhsT, rhs=d2_sb[:, j, :, o:o + w],
                            start=True, stop=True, perf_mode=DR)

                    e = etiles.tile([MT, N], bf16, tag="e", name="e")
                    # ACT: exact exp + rowsums, R0 then R1 head
                    nc.scalar.activation(
                        out=e[:mp, 0:R0], in_=p0[:mp, :], func=Exp,
                        scale=TEMP, accum_out=rsparts[:mp, t:t + 1])
                    nc.scalar.activation(
                        out=e[:mp, R0:AW2], in_=p1t[:mp, 0:AW2 - R0],
                        func=Exp, scale=TEMP,
                        accum_out=rsparts[:mp, NT + t:NT + t + 1])
                    # DVE: Schraudolph exp tail + its rowsum pass
                    nc.vector.tensor_scalar(
                        out=e[:mp, AW2:N].bitcast(i16),
                        in0=p1t[:mp, AW2 - R0:N - R0],
                        scalar1=SCH_S, scalar2=SCH_B, op0=MULT, op1=ADD)
                    nc.vector.tensor_scalar(
                        out=e[:mp, AW2:N], in0=e[:mp, AW2:N],
                        scalar1=1.0, scalar2=None, op0=MULT, op1=ADD,
                        accum_out=rsparts[:mp, 2 * NT + t:2 * NT + t + 1])
                    # DVE: colacc adds per region
                    nc.vector.tensor_add(
                        colacc[:mp, 0:R0], colacc[:mp, 0:R0], e[:mp, 0:R0])
                    nc.vector.tensor_add(
                        colacc[:mp, R0:N], colacc[:mp, R0:N], e[:mp, R0:N])

            # ---- finalize ----
            rowsums = small.tile([MT, NT], f32, name="rowsums")
            nc.vector.tensor_add(
                rowsums, rsparts[:, 0:NT], rsparts[:, NT:2 * NT])
            nc.vector.tensor_add(
                rowsums, rowsums, rsparts[:, 2 * NT:3 * NT])
            rl = small.tile([MT, NT], f32, name="rl")
            nc.scalar.activation(out=rl, in_=rowsums, func=Ln,
                                 accum_out=fin[:, 1:2])

            with tc.tile_pool(name="psF", bufs=1, space="PSUM") as psF:
                # column sums: ones-matmuls into one 7-bank PSUM strip,
                # then a single Ln whose accum_out is sum(log(colsum)).
                csum = psF.tile([1, 3584], f32, tag="csum", name="csum")
                for (off, w) in _mm_chunks(N):
                    nc.tensor.matmul(csum[0:1, off:off + w], lhsT=ones_bf,
                                     rhs=colacc[:, off:off + w],
                                     start=True, stop=True)
                cl = small.tile([1, N], f32, name="cl")
                clsum = small.tile([1, 1], f32, name="clsum")
                nc.scalar.activation(out=cl, in_=csum[0:1, 0:N], func=Ln,
                                     accum_out=clsum)

                # partition-reduce diag and row-log partials in one matmul
                dr_ps = psF.tile([1, 2], f32, tag="drps", name="dr_ps")
                nc.tensor.matmul(dr_ps[0:1, 0:2], lhsT=ones_f32,
                                 rhs=fin[:, 0:2], start=True, stop=True)

                outsb = small.tile([1, 3], f32, name="outsb")
                nc.vector.tensor_copy(outsb[0:1, 0:2], dr_ps[0:1, 0:2])
                nc.vector.tensor_copy(outsb[0:1, 2:3], clsum)
                nc.sync.dma_start(out=out[:, :], in_=outsb)

    nc.compile()
    return nc


def _get_program():
    if "nc" not in _prog_cache:
        _prog_cache["nc"] = _build_program()
    return _prog_cache["nc"]


def _prep_in_maps(inputs):
    p1 = np.asarray(inputs["p1"], dtype=np.float32)
    p2 = np.asarray(inputs["p2"], dtype=np.float32)
    y1 = np.asarray(inputs["y1"]).astype(np.int64)
    x1 = np.asarray(inputs["x1"]).astype(np.int64)
    y2 = np.asarray(inputs["y2"]).astype(np.int64)
    x2 = np.asarray(inputs["x2"]).astype(np.int64)

    des1 = p1[:, :, y1, x1]                      # [B, C, N] f32
    des2 = p2[:, :, y2, x2]
    # DoubleRow pair layouts with small strides:
    # lhsT blocks [128, NT, 2, 128]; rhs blocks [128, NB, 2, 512]
    padm = np.zeros((B, C, MP_PAD - N), np.float32)
    dd1 = np.concatenate([des1, padm], axis=2).reshape(B, 2, MT, NT, MT)
    l1 = dd1.transpose(0, 2, 3, 1, 4)
    padn = np.zeros((B, C, NB * 512 - N), np.float32)
    dd2 = np.concatenate([des2, padn], axis=2).reshape(B, 2, MT, NB, 512)
    r2 = dd2.transpose(0, 2, 3, 1, 4)
    f8_1 = np.ascontiguousarray(l1).astype(ml_dtypes.float8_e4m3fn)
    f8_2 = np.ascontiguousarray(r2).astype(ml_dtypes.float8_e4m3fn)
    # bf16 copies for diag: [128, 2, N] flattened
    dr1 = des1.reshape(B, 2, MT, N).transpose(0, 2, 1, 3)
    dr2 = des2.reshape(B, 2, MT, N).transpose(0, 2, 1, 3)
    bf_1 = np.ascontiguousarray(dr1.reshape(B, MT, 2 * N)).astype(
        ml_dtypes.bfloat16)
    bf_2 = np.ascontiguousarray(dr2.reshape(B, MT, 2 * N)).astype(
        ml_dtypes.bfloat16)
    return [
        {"d1": f8_1[b], "d2": f8_2[b], "b1": bf_1[b], "b2": bf_2[b]}
        for b in range(B)
    ]


def _combine(results):
    total = 0.0
    for b in range(B):
        d, r, c = (float(v) for v in
                   np.asarray(results[b]["out"], dtype=np.float64).ravel())
        total += 2.0 * TEMP * d - r - c
    return np.float32(-total / (B * N))


def kernel(**inputs) -> np.ndarray:
    from concourse.bass_utils import run_bass_kernel_spmd

    nc = _get_program()
    in_maps = _prep_in_maps(inputs)
    res = run_bass_kernel_spmd(nc, in_maps, list(range(B)))
    return _combine(res.results)


# revision 16
# speedup vs baseline: 1.0738x; 1.0738x over previous
"""Trainium2 Bass kernel for the DescriptorLoss dual-softmax loss.

Math (per batch element b):
    des1 = p1[b][:, y1, x1]            # [C=256, N=3540]
    des2 = p2[b][:, y2, x2]            # [C, N]
    dist = TEMP * des1.T @ des2        # [N, N]
    loss_b = 2*trace(dist) - sum_m lse_row[m] - sum_n lse_col[n]
    loss   = -(sum_b loss_b) / (B*N)

Key identities used on-device:
    trace(dist)   = TEMP * <des1, des2>_Frobenius  (elementwise, no matmul)
    lse (no max-subtraction) is safe: |dist| <~ 20, exp fits fp32/bf16 range.

Sharding: data-parallel over the batch dim, one batch element per
NeuronCore (B == 8 == n_cores).  The host gathers descriptors with the
int32 index arrays (pure data movement), casts to bf16, runs the SPMD
program, and averages the 8 per-core partial sums.

Per-core engine assignment:
    PE : dist tiles (bf16 inputs, fp32 PSUM accumulate over C=2x128)
         + ones-matmul partition reductions at the end
    ACT: exp(TEMP*dist) PSUM->SBUF(bf16), accum_out = row sums (free!)
         + final Ln (with accum_out again for the sum of logs)
    DVE: column-sum accumulation in bf16 (2x mode) + diag term via
         scalar_tensor_tensor's accum_out

Pipeline: each m-tile's 3540 dist columns live in two PSUM regions
(1770+1770 fp32 = 4+4 banks).  PE refills a region only after ACT
finished exp-ing it (WAR tracked at byte granularity); each refill fits
inside ACT's work on the other region, so the steady state is ACT-bound
at ~3.9us per m-tile.
"""

import numpy as np
import ml_dtypes

B = 8
C = 256
N = 3540
TEMP = 0.2
KP = 128           # contraction chunk (partition dim)
NK = C // KP       # 2
MT = 128           # rows per m-tile
N_MTILES = (N + MT - 1) // MT   # 28 (last tile has 84 rows)
REGIONS = [(0, 1770), (1770, N - 1770)]   # 4+4 PSUM banks (chunks stay in-bank)
MM_N = 512         # max moving free dim per matmul
HEAD = 512         # fast-start column split

_prog_cache = {}


def _mm_chunks(width):
    out = []
    off = 0
    while off < width:
        w = min(MM_N, width - off)
        out.append((off, w))
        off += w
    return out


def _build_program():
    import contextlib
    import concourse.bacc as bacc
    import concourse.tile as tile
    from concourse import mybir

    dt = mybir.dt
    f32 = dt.float32
    bf16 = dt.bfloat16
    Exp = mybir.ActivationFunctionType.Exp
    Ln = mybir.ActivationFunctionType.Ln
    MULT = mybir.AluOpType.mult

    nc = bacc.Bacc(
        "TRN2", target_bir_lowering=False, debug=False, num_devices=B)
    d1 = nc.dram_tensor("d1", [C, N], bf16, kind="ExternalInput")
    d2 = nc.dram_tensor("d2", [C, N], bf16, kind="ExternalInput")
    out = nc.dram_tensor("out", [1, 3], f32, kind="ExternalOutput")

    with tile.TileContext(nc) as tc:
        with (
            tc.tile_pool(name="persist", bufs=1) as persist,
            tc.tile_pool(name="etiles", bufs=2) as etiles,
            tc.tile_pool(name="small", bufs=1) as small,
        ):
            # ---- load descriptors (bf16, [128, N] per C-chunk) ----
            # Split + ordered so tile 0's operands land first, spread over
            # two HWDGE queues (sync + scalar).
            d1_sb = [persist.tile([KP, N], bf16, tag=f"d1_{k}", name=f"d1_{k}")
                     for k in range(NK)]
            d2_sb = [persist.tile([KP, N], bf16, tag=f"d2_{k}", name=f"d2_{k}")
                     for k in range(NK)]
            # Each [128, W] piece costs ~128 partition-runs of queue time
            # regardless of W, so: heads (tile 0's first chunks) lead both
            # HWDGE queues, des2 rests (needed during tile 0) follow, and
            # des1 rests (needed only from m-tile 4 on) ride SWDGE.
            for k in range(NK):   # lhsT columns for m-tiles 0..3
                nc.sync.dma_start(out=d1_sb[k][:, 0:HEAD],
                                  in_=d1[k * KP:(k + 1) * KP, 0:HEAD])
            for k in range(NK):   # dist columns for region 0 (+ start of 1)
                nc.scalar.dma_start(out=d2_sb[k][:, 0:2048],
                                    in_=d2[k * KP:(k + 1) * KP, 0:2048])
            for k in range(NK):   # region 2
                nc.sync.dma_start(out=d2_sb[k][:, 2048:N],
                                  in_=d2[k * KP:(k + 1) * KP, 2048:N])
            for k in range(NK):   # rest of des1 (needed only from m-tile 4)
                nc.scalar.dma_start(out=d1_sb[k][:, HEAD:N],
                                    in_=d1[k * KP:(k + 1) * KP, HEAD:N])

            colacc = persist.tile([MT, N], bf16, tag="colacc", name="colacc")
            nc.vector.memset(colacc, 0.0)

            # rsparts[:, r*N_MTILES + t] = rowsum of exp over region r of
            # m-tile t.  0.5-init: rows of the last (84-row) m-tile that do
            # not exist sum to 1.0 -> Ln contributes 0.
            rsparts = small.tile([MT, 2 * N_MTILES], f32, tag="rsparts",
                                 name="rsparts")
            nc.vector.memset(rsparts, 0.5)

            ones_bf = small.tile([KP, 1], bf16, name="ones_bf")
            nc.vector.memset(ones_bf, 1.0)
            ones_f32 = small.tile([KP, 1], f32, name="ones_f32")
            nc.vector.memset(ones_f32, 1.0)

            # fin[:,0] = diag partial, fin[:,1] = sum of row-logs partial
            fin = small.tile([KP, 2], f32, tag="fin", name="fin")

            # ---- diag term: sum(des1 * des2) per partition ----
            # (tensor_tensor_reduce wedges the device; scalar_tensor_tensor
            # with accum_out is the stable fused multiply+rowsum.)
            scratch = persist.tile([KP, N], bf16, tag="scratch", name="scratch")
            diag0 = small.tile([KP, 1], f32, name="diag0")
            diag1 = small.tile([KP, 1], f32, name="diag1")
            nc.vector.scalar_tensor_tensor(
                out=scratch, in0=d1_sb[0], scalar=1.0, in1=d2_sb[0],
                op0=MULT, op1=MULT, accum_out=diag0)
            nc.vector.scalar_tensor_tensor(
                out=scratch, in0=d1_sb[1], scalar=1.0, in1=d2_sb[1],
                op0=MULT, op1=MULT, accum_out=diag1)
            nc.vector.tensor_add(fin[:, 0:1], diag0, diag1)

            # ---- main loop over m-tiles ----
            with contextlib.ExitStack() as psctx:
                pspools = [
                    psctx.enter_context(
                        tc.tile_pool(name=f"ps{r}", bufs=1, space="PSUM"))
                    for r in range(2)
                ]
                for t in range(N_MTILES):
                    m0 = t * MT
                    mp = min(MT, N - m0)
                    ps = [pspools[r].tile([MT, REGIONS[r][1]], f32,
                                          tag=f"ps{r}", name=f"ps{r}")
                          for r in range(2)]
                    # PE: region-outer, k-inner, so each region completes
                    # as early as possible.
                    for r in range(2):
                        g, gw = REGIONS[r]
                        for k in range(NK):
                            for (off, w) in _mm_chunks(gw):
                                nc.tensor.matmul(
                                    ps[r][:mp, off:off + w],
                                    lhsT=d1_sb[k][:, m0:m0 + mp],
                                    rhs=d2_sb[k][:, g + off:g + off + w],
                                    start=(k == 0), stop=(k == NK - 1))

                    # ACT: exp -> bf16 SBUF + rowsum accum; DVE: colacc add.
                    for r in range(2):
                        g, gw = REGIONS[r]
                        e = etiles.tile([MT, gw], bf16, tag=f"e{r}",
                                        name=f"e{r}")
                        nc.scalar.activation(
                            out=e[:mp, :], in_=ps[r][:mp, :], func=Exp,
                            scale=TEMP,
                            accum_out=rsparts[:mp, r * N_MTILES + t:
                                              r * N_MTILES + t + 1])
                        nc.vector.tensor_add(
                            colacc[:mp, g:g + gw],
                            colacc[:mp, g:g + gw],
                            e[:mp, :])

            # ---- finalize ----
            # rowsums; invalid rows = 1.0 -> Ln 0.
            rowsums = small.tile([MT, N_MTILES], f32, tag="rowsums",
                                 name="rowsums")
            nc.vector.tensor_add(
                rowsums, rsparts[:, 0:N_MTILES],
                rsparts[:, N_MTILES:2 * N_MTILES])
            rl = small.tile([MT, N_MTILES], f32, tag="rl", name="rl")
            nc.scalar.activation(out=rl, in_=rowsums, func=Ln,
                                 accum_out=fin[:, 1:2])

            with tc.tile_pool(name="psF", bufs=1, space="PSUM") as psF:
                # column sums: ones-matmuls into one 7-bank PSUM strip,
                # then a single Ln whose accum_out is sum(log(colsum)).
                csum = psF.tile([1, 3584], f32, tag="csum", name="csum")
                for (off, w) in _mm_chunks(N):
                    nc.tensor.matmul(csum[0:1, off:off + w], lhsT=ones_bf,
                                     rhs=colacc[:, off:off + w],
                                     start=True, stop=True)
                cl = small.tile([1, N], f32, tag="cl", name="cl")
                clsum = small.tile([1, 1], f32, tag="clsum", name="clsum")
                nc.scalar.activation(out=cl, in_=csum[0:1, 0:N], func=Ln,
                                     accum_out=clsum)

                # partition-reduce diag and row-log partials in one matmul
                dr_ps = psF.tile([1, 2], f32, tag="drps", name="dr_ps")
                nc.tensor.matmul(dr_ps[0:1, 0:2], lhsT=ones_f32,
                                 rhs=fin[:, 0:2], start=True, stop=True)

                outsb = small.tile([1, 3], f32, tag="outsb", name="outsb")
                nc.vector.tensor_copy(outsb[0:1, 0:2], dr_ps[0:1, 0:2])
                nc.vector.tensor_copy(outsb[0:1, 2:3], clsum)
                nc.sync.dma_start(out=out[:, :], in_=outsb)

    nc.compile()
    return nc


def _get_program():
    if "nc" not in _prog_cache:
        _prog_cache["nc"] = _build_program()
    return _prog_cache["nc"]


def _prep_in_maps(inputs):
    p1 = np.asarray(inputs["p1"], dtype=np.float32)
    p2 = np.asarray(inputs["p2"], dtype=np.float32)
    y1 = np.asarray(inputs["y1"]).astype(np.int64)
    x1 = np.asarray(inputs["x1"]).astype(np.int64)
    y2 = np.asarray(inputs["y2"]).astype(np.int64)
    x2 = np.asarray(inputs["x2"]).astype(np.int64)

    # Host-side gather (data movement only): [B, C, N] then bf16 cast.
    des1 = p1[:, :, y1, x1].astype(ml_dtypes.bfloat16)
    des2 = p2[:, :, y2, x2].astype(ml_dtypes.bfloat16)
    return [
        {"d1": np.ascontiguousarray(des1[b]),
         "d2": np.ascontiguousarray(des2[b])}
        for b in range(B)
    ]


def kernel(**inputs) -> np.ndarray:
    from concourse.bass_utils import run_bass_kernel_spmd

    nc = _get_program()
    in_maps = _prep_in_maps(inputs)
    res = run_bass_kernel_spmd(nc, in_maps, list(range(B)))
    total = 0.0
    for b in range(B):
        d, r, c = (float(v) for v in np.asarray(res.results[b]["out"]).ravel())
        total += 2.0 * TEMP * d - r - c
    loss = -total / (B * N)
    return np.float32(loss)
